# revision 2
# baseline (speedup 1.0000x reference)
"""Trainium2 Bass kernel for EquivariantGraphConv message passing.

Strategy (8 NeuronCores, SPMD single NEFF):
  - Nodes sharded 12544/core. Each core computes its h = x@W_node + b_node
    shard on the PE, then an AllGather replicates h into every core's HBM.
  - Edges sharded by destination core. Host sorts each core's edges into
    "rounds" (round j = the j-th incoming edge of each destination), so every
    dma_scatter_add instruction has unique destination rows (the SDMA CCE
    read-modify-write races on duplicate rows within one instruction).
    Rounds alternate between two accumulator tables so consecutive rounds
    pipeline; chained same-table scatters are ordered by Tile's WAW deps.
  - Within a round, tokens are grouped by source-node quadrant (dma_gather
    indexes are int16, so the 100K-row h table is addressed 32768 rows at a
    time) and gathered with hardware dma_gather straight from the replicated
    h table in HBM.
  - e = edge_attr @ W_edge + b_edge runs on the PE per 128-token chunk
    (K=33 with a ones-row folding in the bias), msg = h_gather + e on the DVE,
    and a constant ones column rides along as the scatter's count channel.
  - Finally out = s / max(cnt, 1) per 128-row chunk, written as the core's
    output shard; the host concatenates shards.
"""

import numpy as np

N_CORES = 8
NL = 12544                 # nodes per core (uniform, 100000 padded to 100352)
NPAD = NL * N_CORES
QBITS = 15                 # gather quadrant = node >> 15 (int16 index limit)
CELLCAP = 2048             # max tokens per dma_gather (16 chunks)
SPLIT = 8064               # max tokens per dma_scatter_add (ring capacity)
IN_CH, OUT_CH, EDGE_DIM = 128, 64, 32


# ---------------------------------------------------------------- host plan

def _build_plan(edge_index):
    row = np.asarray(edge_index[0], dtype=np.int64)
    col = np.asarray(edge_index[1], dtype=np.int64)
    core = row // NL

    per_core_raw = []
    R = 0
    for c in range(N_CORES):
        m = np.nonzero(core == c)[0]
        r_l = (row[m] - c * NL).astype(np.int64)
        cc = col[m]
        order = np.argsort(r_l, kind="stable")
        sd = r_l[order]
        if sd.size:
            starts = np.r_[0, np.nonzero(np.diff(sd))[0] + 1]
            lens = np.diff(np.r_[starts, sd.size])
            occ = np.arange(sd.size) - np.repeat(starts, lens)
            rnd = np.empty_like(occ)
            rnd[order] = occ
            R = max(R, int(occ.max()) + 1)
        else:
            rnd = np.zeros(0, np.int64)
        per_core_raw.append((m, r_l, cc, rnd, cc >> QBITS))

    counts = np.zeros((N_CORES, R, 4), np.int64)
    for c in range(N_CORES):
        m, r_l, cc, rnd, quad = per_core_raw[c]
        if rnd.size:
            np.add.at(counts[c], (rnd, quad), 1)
    gmax = counts.max(axis=0)
    csz = ((gmax + 127) // 128) * 128

    cells = []
    tok = 0
    round_span = []
    for r in range(R):
        r0 = tok
        for q in range(4):
            s = int(csz[r, q])
            if s == 0:
                continue
            cells.append((r, q, s, tok))
            tok += s
        round_span.append((r0, tok - r0))
    TOK = tok

    per_core = []
    junk_needed = 0
    for c in range(N_CORES):
        m, r_l, cc, rnd, quad = per_core_raw[c]
        gidx = np.zeros(TOK, np.int16)
        sidx = np.zeros(TOK, np.int16)
        perm = np.full(TOK, -1, np.int64)
        key = rnd * 4 + quad
        ordk = np.lexsort((cc, key))
        sk = key[ordk]
        bounds = np.searchsorted(sk, np.arange(R * 4 + 1))
        for r, q, size, off in cells:
            a, b = bounds[r * 4 + q], bounds[r * 4 + q + 1]
            sel = ordk[a:b]
            n = sel.size
            gidx[off:off + n] = (cc[sel] & ((1 << QBITS) - 1)).astype(np.int16)
            sidx[off:off + n] = r_l[sel].astype(np.int16)
            perm[off:off + n] = m[sel]
        for r0_, span in round_span:
            pad_pos = np.nonzero(perm[r0_:r0_ + span] == -1)[0]
            junk_needed = max(junk_needed, pad_pos.size)
            sidx[r0_ + pad_pos] = (NL + np.arange(pad_pos.size)).astype(np.int16)
        per_core.append({"gidx": gidx, "sidx": sidx, "perm": perm})

    trows = NL + ((max(junk_needed, 1) + 127) // 128) * 128
    assert trows <= 32767
    return {"cells": cells, "round_span": round_span, "R": R, "TOK": TOK,
            "per_core": per_core, "trows": trows}


def _wrap_rep(idx):
    w = idx.reshape(-1, 16).T.copy()
    return np.ascontiguousarray(np.tile(w, (8, 1)))


def _pack_inputs(plan, x, edge_attr, W_node, b_node, W_edge, b_edge):
    TOK = plan["TOK"]
    n = x.shape[0]
    xpad = np.zeros((NPAD, IN_CH), np.float32)
    xpad[:n] = np.asarray(x, np.float32)
    Wext = np.concatenate(
        [np.asarray(W_edge, np.float32), np.asarray(b_edge, np.float32)[None, :]],
        axis=0)
    in_maps = []
    for c in range(N_CORES):
        pc = plan["per_core"][c]
        perm = pc["perm"]
        attrT = np.zeros((EDGE_DIM + 1, TOK), np.float32)
        real = perm >= 0
        attrT[:EDGE_DIM, real] = np.asarray(edge_attr, np.float32)[perm[real]].T
        attrT[EDGE_DIM, :] = 1.0
        in_maps.append({
            "xT": np.ascontiguousarray(xpad[c * NL:(c + 1) * NL].T),
            "W_node": np.ascontiguousarray(np.asarray(W_node, np.float32)),
            "b_node": np.ascontiguousarray(np.asarray(b_node, np.float32)[None, :]),
            "W_ext": np.ascontiguousarray(Wext),
            "attrT": attrT,
            "gidx": _wrap_rep(pc["gidx"]),
            "sidx": _wrap_rep(pc["sidx"]),
        })
    return in_maps


# ---------------------------------------------------------------- device IR

def _build_nc(plan):
    import sys
    if "/opt/trn_rl_repo" not in sys.path:
        sys.path.insert(0, "/opt/trn_rl_repo")
    from concourse import bass, mybir, bacc, tile

    f32 = mybir.dt.float32
    i16 = mybir.dt.int16
    TOK = plan["TOK"]
    trows = plan["trows"]
    cells = plan["cells"]
    round_span = plan["round_span"]
    crmax = max(s for _, s in round_span) // 128

    nc = bacc.Bacc("TRN2", target_bir_lowering=False, debug=False,
                   num_devices=N_CORES)

    xT = nc.dram_tensor("xT", [IN_CH, NL], f32, kind="ExternalInput")
    Wn_d = nc.dram_tensor("W_node", [IN_CH, OUT_CH], f32, kind="ExternalInput")
    bn_d = nc.dram_tensor("b_node", [1, OUT_CH], f32, kind="ExternalInput")
    We_d = nc.dram_tensor("W_ext", [EDGE_DIM + 1, OUT_CH], f32, kind="ExternalInput")
    at_d = nc.dram_tensor("attrT", [EDGE_DIM + 1, TOK], f32, kind="ExternalInput")
    gi_d = nc.dram_tensor("gidx", [128, TOK // 16], i16, kind="ExternalInput")
    si_d = nc.dram_tensor("sidx", [128, TOK // 16], i16, kind="ExternalInput")
    out_d = nc.dram_tensor("out", [NL, OUT_CH], f32, kind="ExternalOutput")

    ts = bass.ts

    with tile.TileContext(nc) as tc:
        with (
            tc.tile_pool(name="dram", bufs=1, space="DRAM") as dram,
            tc.tile_pool(name="const", bufs=1) as cpool,
            tc.tile_pool(name="ph1", bufs=2) as hpool,
            tc.tile_pool(name="psum", bufs=2, space="PSUM") as ppool,
            tc.tile_pool(name="msgp", bufs=2) as mpool,
            tc.tile_pool(name="gat", bufs=2) as gpool,
            tc.tile_pool(name="idx", bufs=2) as ipool,
            tc.tile_pool(name="fin", bufs=2) as fpool,
        ):
            h_shard = dram.tile([NL, OUT_CH], f32)
            h_full = dram.tile([NPAD, OUT_CH], f32)
            tabs = [dram.tile([trows, 128], f32, tag=f"tab{i}", name=f"tab{i}")
                    for i in range(2)]

            # constants
            wn = cpool.tile([IN_CH, OUT_CH], f32)
            bn = cpool.tile([1, OUT_CH], f32)
            we = cpool.tile([EDGE_DIM + 1, OUT_CH], f32)
            ones1 = cpool.tile([1, 128], f32)
            zini = cpool.tile([128, 2048], f32)
            nc.sync.dma_start(wn[:], Wn_d[:])
            nc.sync.dma_start(bn[:], bn_d[:])
            nc.sync.dma_start(we[:], We_d[:])
            nc.vector.memset(ones1[:], 1.0)
            nc.vector.memset(zini[:], 0.0)

            # zero the accumulator tables
            for t in tabs:
                r0 = 0
                while r0 < trows:
                    rn = min(2048, trows - r0)
                    nc.sync.dma_start(t[r0:r0 + rn, :], zini[:, :rn])
                    r0 += rn

            # phase 1: h = x @ W_node + b_node for the local shard
            for k in range(NL // 128):
                xt = hpool.tile([IN_CH, 128], f32, tag="xt")
                nc.sync.dma_start(xt[:], xT[:, ts(k, 128)])
                hp = ppool.tile([128, OUT_CH], f32, tag="hps")
                nc.tensor.matmul(hp[:], xt[:], wn[:], start=True, stop=False)
                nc.tensor.matmul(hp[:], ones1[:], bn[:], start=False, stop=True)
                hs = hpool.tile([128, OUT_CH], f32, tag="hsb")
                nc.scalar.copy(hs[:], hp[:])
                nc.sync.dma_start(h_shard[ts(k, 128), :], hs[:])

            nc.gpsimd.collective_compute(
                "AllGather",
                mybir.AluOpType.bypass,
                replica_groups=[list(range(N_CORES))],
                ins=[h_shard.opt()],
                outs=[h_full.opt()],
            )

            # quadrant base views of the replicated h table
            qviews = []
            for q in range(4):
                lo = q << QBITS
                hi = min(lo + (1 << QBITS), NPAD)
                qviews.append(h_full[lo:hi, :])

            # main loop over rounds
            cell_by_round = {}
            for r, q, size, off in cells:
                cell_by_round.setdefault(r, []).append((q, size, off))

            for r, (r0, span) in enumerate(round_span):
                cr = span // 128
                msg = mpool.tile([128, cr, OUT_CH + 1], f32, tag="msg")
                nc.vector.memset(msg[:, :, OUT_CH:OUT_CH + 1], 1.0)
                si = ipool.tile([128, span // 16], i16, tag="si")
                nc.sync.dma_start(si[:], si_d[:, r0 // 16:(r0 + span) // 16])

                for q, size, off in cell_by_round[r]:
                    s0 = 0
                    while s0 < size:
                        sub = min(CELLCAP, size - s0)
                        t0 = off + s0            # global token offset
                        subc = sub // 128
                        gi = ipool.tile([128, sub // 16], i16, tag="gi")
                        nc.sync.dma_start(gi[:], gi_d[:, t0 // 16:(t0 + sub) // 16])
                        gt = gpool.tile([128, subc, OUT_CH], f32, tag="gath")
                        nc.gpsimd.dma_gather(
                            gt[:], qviews[q], gi[:],
                            num_idxs=sub, num_idxs_reg=sub, elem_size=OUT_CH,
                            single_packet=False)
                        at = gpool.tile([EDGE_DIM + 1, sub], f32, tag="attr")
                        nc.sync.dma_start(at[:], at_d[:, t0:t0 + sub])
                        ep = ppool.tile([128, subc, OUT_CH], f32, tag="eps")
                        for j in range(subc):
                            nc.tensor.matmul(ep[:, j, :], at[:, ts(j, 128)], we[:],
                                             start=True, stop=True)
                        c0 = (t0 - r0) // 128
                        nc.vector.tensor_add(
                            msg[:, c0:c0 + subc, :OUT_CH], ep[:], gt[:])
                        s0 += sub

                # scatter the round into its parity table
                tab = tabs[r % 2]
                a = 0
                while a < cr:
                    b = min(a + SPLIT // 128, cr)
                    ntok = (b - a) * 128
                    nc.gpsimd.dma_scatter_add(
                        tab[:, 0:OUT_CH + 1], msg[:, a:b, :],
                        si[:, a * 8:b * 8],
                        num_idxs=ntok, num_idxs_reg=ntok,
                        elem_size=OUT_CH + 1, elem_step=128,
                        single_packet=False)
                    a = b

            # final: out = (A + B)[:, :64] / max((A + B)[:, 64], 1)
            for k in range(NL // 128):
                ta = fpool.tile([128, OUT_CH + 1], f32, tag="fa")
                tb = fpool.tile([128, OUT_CH + 1], f32, tag="fb")
                nc.sync.dma_start(ta[:], tabs[0][ts(k, 128), 0:OUT_CH + 1])
                nc.sync.dma_start(tb[:], tabs[1][ts(k, 128), 0:OUT_CH + 1])
                nc.vector.tensor_add(ta[:], ta[:], tb[:])
                cm = fpool.tile([128, 2], f32, tag="fc")
                nc.vector.tensor_scalar_max(cm[:, 0:1], ta[:, OUT_CH:OUT_CH + 1], 1.0)
                nc.vector.reciprocal(cm[:, 1:2], cm[:, 0:1])
                ob = fpool.tile([128, OUT_CH], f32, tag="fo")
                nc.vector.tensor_scalar_mul(ob[:], ta[:, 0:OUT_CH], cm[:, 1:2])
                nc.sync.dma_start(out_d[ts(k, 128), :], ob[:])

    nc.compile()
    return nc


# ---------------------------------------------------------------- entry

_CACHE = {}


def _get_compiled(edge_index_key, edge_index):
    if edge_index_key not in _CACHE:
        plan = _build_plan(edge_index)
        nc = _build_nc(plan)
        _CACHE[edge_index_key] = (plan, nc)
    return _CACHE[edge_index_key]


def kernel(x, edge_index, edge_attr, W_node, b_node, W_edge, b_edge):
    import sys
    if "/opt/trn_rl_repo" not in sys.path:
        sys.path.insert(0, "/opt/trn_rl_repo")
    from concourse.bass_utils import run_bass_kernel_spmd

    x = np.asarray(x)
    edge_index = np.asarray(edge_index)
    n = x.shape[0]

    key = hash(edge_index.tobytes())
    plan, nc = _get_compiled(key, edge_index)
    in_maps = _pack_inputs(plan, x, edge_attr, W_node, b_node, W_edge, b_edge)
    res = run_bass_kernel_spmd(nc, in_maps, core_ids=list(range(N_CORES)))
    out = np.concatenate([res.results[c]["out"] for c in range(N_CORES)], axis=0)
    return np.ascontiguousarray(out[:n])
